# revision 1
# baseline (speedup 1.0000x reference)
"""Trainium2 Bass kernel for causal self-attention with sliding window.

Reference computation (per batch b):
    qkv = W_qkv @ x + b_qkv            # [3C, T], C=256, T=4096
    q,k,v -> [H=4, D=64, T]
    attn = softmax(mask(q^T k / sqrt(D)))   # causal, window 1024
    y = attn @ v                       # [H, D, T] -> [C, T]
    out = x + W_proj @ y + b_proj

Sharding: 8 cores = 2 batches x 4 time-chunks of 1024 queries. Each core
gets x for its chunk plus a 1024-step halo (for keys/values), computes all
4 heads and the full output projection locally, and writes out[256, 1024].
Unshard is pure concatenation.

On-core algorithm (flash-style, no transposes):
  - scores are computed TRANSPOSED: sT[s, t] = sum_d k[d,s] q[d,t] so that
    the PV matmul (contraction over s) consumes exp(sT) directly.
  - biases for q/k enter via an extra "kvalid" row appended to x (value 1
    for real timesteps, 0 for halo padding of the first chunk); the matching
    weight row holds b_qkv. Padded keys therefore get k=0 -> score 0 ->
    exp()=1; these bogus contributions are counted (npad) and subtracted
    from the softmax denominator. v gets no bias on-chip: a softmax-weighted
    average passes the v-bias through additively, so it is folded into the
    output-projection bias on the host (b_eff = b_proj + W_proj @ b_v).
  - causal masks: only two triangular 128x128 blocks per 256-query block
    need element masks (multiplicative, post-exp); edge key chunks are
    computed at N=128 with their dead halves memset to zero so the PV
    accumulation runs uniform full-rate N=256 float32r matmuls.
  - softmax denominator Z arrives for free as an extra ones-column in the
    PV stationary operand; 1/Z is broadcast back over the d-partitions via
    small matmuls against an identity matrix.
"""

import math
import sys

import numpy as np

if "/opt/trn_rl_repo" not in sys.path:
    sys.path.insert(0, "/opt/trn_rl_repo")

B = 2
C = 256
T = 4096
H = 4
D = 64
WINDOW = 1024
TCH = 1024          # queries per core
TLOC = 2048         # halo + chunk
NQB = TCH // 128    # 128-query sub-blocks per core (Z bookkeeping)
NVCH = TLOC // 128  # 16 v chunks
VSTR = 4 * 66       # v chunk stride: 4 heads x (64 d + 1 ones + 1 pad)


SCL = 1.0 / math.sqrt(D)

# blob layouts: name -> (width); offsets accumulate in order
_B1LAYOUT = [("we0", 512), ("we1", 512), ("xvw", TLOC + 512),
             ("xe0h", 1024), ("xe1h", 1024)]
_B2LAYOUT = [("xe0l", 1024), ("xe1l", 1024), ("wv0", VSTR), ("wv1", VSTR),
             ("wp0", C), ("wp1", C)]
_B3LAYOUT = [("t12", 512), ("npadT", NQB), ("ident", 128), ("beff", 2)]


def _offsets(layout):
    offs, c = {}, 0
    for nm, w in layout:
        offs[nm] = c
        c += w
    return offs, c


_B1OFF, NB1 = _offsets(_B1LAYOUT)
_B2OFF, NB2 = _offsets(_B2LAYOUT)
_B3OFF, NB3 = _offsets(_B3LAYOUT)


def _build_nc():
    import concourse.bass as bass
    import concourse.bacc as bacc
    import concourse.tile as tile
    from concourse import mybir

    f32 = mybir.dt.float32
    f32r = mybir.dt.float32r
    Alu = mybir.AluOpType
    Act = mybir.ActivationFunctionType

    def R(ap):
        # operands are declared float32r natively; kept for call-site clarity
        return ap

    nc = bacc.Bacc("TRN2", debug=False, target_bir_lowering=False, num_devices=8)

    blob1 = nc.dram_tensor("blob1", [128, NB1], f32r, kind="ExternalInput").ap()
    blob2 = nc.dram_tensor("blob2", [128, NB2], f32r, kind="ExternalInput").ap()
    blob3 = nc.dram_tensor("blob3", [128, NB3], f32, kind="ExternalInput").ap()
    out = nc.dram_tensor("out", [C, TCH], f32, kind="ExternalOutput").ap()

    with tile.TileContext(nc) as tc:
        with tc.tile_pool(name="singles", bufs=1) as singles:
            # ---- two blob loads; everything else is views into them ----
            b1 = singles.tile([128, NB1], f32r)
            b2 = singles.tile([128, NB2], f32r)
            cw = _B1OFF["xvw"]
            c1 = _B1OFF["xe0h"]
            nc.sync.dma_start(out=b1[:, 0:cw], in_=blob1[:, 0:cw])
            # kvalid + qkv-bias share row 0: only 1 of 128 rows carries data
            nc.sync.dma_start(out=b1[0:1, cw:c1], in_=blob1[0:1, cw:c1])
            nc.sync.dma_start(out=b1[:, c1:NB1], in_=blob1[:, c1:NB1])
            c2 = _B2OFF["wv0"]
            nc.sync.dma_start(out=b2[:, 0:c2], in_=blob2[:, 0:c2])
            nc.sync.dma_start(out=b2[:, c2:NB2], in_=blob2[:, c2:NB2])
            b3 = singles.tile([128, NB3], f32)
            nc.sync.dma_start(out=b3, in_=blob3)
            o = _B1OFF
            we0 = b1[:, o["we0"]:o["we0"] + 512]
            we1 = b1[:, o["we1"]:o["we1"] + 512]
            xv = b1[0:1, o["xvw"]:o["xvw"] + TLOC]
            wrow = b1[0:1, o["xvw"] + TLOC:o["xvw"] + TLOC + 512]
            xe_hi = [b1[:, o["xe0h"]:o["xe0h"] + 1024],
                     b1[:, o["xe1h"]:o["xe1h"] + 1024]]
            o2 = _B2OFF
            xe_lo = [b2[:, o2["xe0l"]:o2["xe0l"] + 1024],
                     b2[:, o2["xe1l"]:o2["xe1l"] + 1024]]
            wv0 = b2[:, o2["wv0"]:o2["wv0"] + VSTR]
            wv1 = b2[:, o2["wv1"]:o2["wv1"] + VSTR]
            wp0 = b2[:, o2["wp0"]:o2["wp0"] + C]
            wp1 = b2[:, o2["wp1"]:o2["wp1"] + C]
            o3 = _B3OFF
            t12_s = b3[:, o3["t12"]:o3["t12"] + 512]
            npad_s = b3[:, o3["npadT"]:o3["npadT"] + NQB]
            idn = b3[:, o3["ident"]:o3["ident"] + 128]
            beff_s = b3[:, o3["beff"]:o3["beff"] + 2]

            def xe(m, c0, w):
                # x slice helper across the hi(b1)/lo(b2) split
                if c0 >= 1024:
                    return xe_hi[m][:, c0 - 1024:c0 - 1024 + w]
                assert c0 + w <= 1024
                return xe_lo[m][:, c0:c0 + w]

            ones64 = singles.tile([128, D], f32)
            nc.vector.memset(ones64, 1.0)
            one11 = singles.tile([1, 1], f32)
            nc.vector.memset(one11, 1.0)

            # persistent activation buffers
            qs = [singles.tile([128, TCH], f32r, name=f"qs{i}") for i in range(2)]
            ks = [singles.tile([128, TLOC], f32r, name=f"ks{i}") for i in range(2)]
            v_sb = singles.tile([128, NVCH * VSTR], f32r)
            ya = [singles.tile([128, TCH], f32r, name=f"ya{i}") for i in range(2)]
            ot = [singles.tile([128, TCH], f32, name=f"ot{i}") for i in range(2)]


            # ---- phase 1: q/k projections (bias via kvalid row) ----
            with tc.tile_pool(name="proj", bufs=4, space="PSUM") as proj:
                for m in range(2):
                    for n in range(2):  # q: x cols 1024 + 512n
                        ps = proj.tile([128, 512], f32, tag="proj")
                        c0 = TCH + n * 512
                        nc.tensor.matmul(ps, R(we0[:, m * 128:(m + 1) * 128]),
                                         R(xe(0, c0, 512)), start=True, stop=False)
                        nc.tensor.matmul(ps, R(we1[:, m * 128:(m + 1) * 128]),
                                         R(xe(1, c0, 512)), start=False, stop=False)
                        nc.tensor.matmul(ps, R(wrow[:, m * 128:(m + 1) * 128]),
                                         R(xv[:, c0:c0 + 512]), start=False, stop=True)
                        nc.scalar.copy(qs[m][:, n * 512:(n + 1) * 512], ps)
                    for n in range(4):  # k: all 2048 cols
                        ps = proj.tile([128, 512], f32, tag="proj")
                        c0 = n * 512
                        w0 = C + m * 128
                        nc.tensor.matmul(ps, R(we0[:, w0:w0 + 128]),
                                         R(xe(0, c0, 512)), start=True, stop=False)
                        nc.tensor.matmul(ps, R(we1[:, w0:w0 + 128]),
                                         R(xe(1, c0, 512)), start=False, stop=False)
                        nc.tensor.matmul(ps, R(wrow[:, w0:w0 + 128]),
                                         R(xv[:, c0:c0 + 512]), start=False, stop=True)
                        nc.vector.tensor_copy(ks[m][:, n * 512:(n + 1) * 512], ps)

                # ---- phase 2: v projection, transposed ([s, c] chunks) ----
                for jc in range(NVCH):
                    ps = proj.tile([128, VSTR], f32, tag="projv")
                    c0 = jc * 128
                    nc.tensor.matmul(ps, R(xe(0, c0, 128)), R(wv0),
                                     start=True, stop=False)
                    nc.tensor.matmul(ps, R(xe(1, c0, 128)), R(wv1),
                                     start=False, stop=True)
                    nc.vector.tensor_copy(v_sb[:, jc * VSTR:(jc + 1) * VSTR], ps)
            # ones columns for the Z row of PV (column 64 of each head
            # block) — one memset per chunk so PV never waits on chunks it
            # doesn't read (a full-tile memset would barrier all of v proj)
            for jc in range(NVCH):
                vo = bass.AP(
                    tensor=v_sb.tensor, offset=v_sb.offset + jc * VSTR + 64,
                    ap=[v_sb.ap[0], [66, 4], [1, 1]],
                )
                nc.gpsimd.memset(vo.bitcast(f32), 1.0)

            # ---- phase 3: attention ----
            # PSUM budget (8 banks): sp2 pool 2x2 + spc pool 2x1 + yps 1x2.
            # Scores for a 256-query block live in three tiles:
            #   spA [128,1024]: c0-live[0:128] dead[128:256] c1[256:512]
            #                   c2[512:768] c3[768:1024]
            #   spB [128,1024]: c4..c7
            #   spC [128, 512]: c8[0:256] dead[256:384] c9-live[384:512]
            # Z is extracted per query block so the per-head normalization
            # chain overlaps the next block/head's score pipeline.
            with tc.tile_pool(name="sp2p", bufs=2, space="PSUM") as sp2p, \
                 tc.tile_pool(name="sp1p", bufs=2, space="PSUM") as sp1p, \
                 tc.tile_pool(name="ypp", bufs=1, space="PSUM") as ypp, \
                 tc.tile_pool(name="epp", bufs=6) as epp, \
                 tc.tile_pool(name="zp", bufs=2) as zp:
                for h in range(H):
                    ti = h // 2
                    rb = (h % 2) * 64
                    kt, qt = ks[ti], qs[ti]
                    yph = [ypp.tile([65, TCH // 2], f32, tag="yps",
                                    name=f"yph{i}", bufs=2) for i in range(2)]
                    zrow = zp.tile([1, TCH], f32, tag="zrow", name="zrow")
                    ysb = zp.tile([64, TCH], f32, tag="ysb", name="ysb")
                    zcolsb = zp.tile([128, NQB], f32, tag="zcolsb", name="zcolsb")
                    for qb in range(4):  # 256-query blocks
                        q0 = qb * 256
                        # spA: c1[0:256] c2[256:512] c3[512:768]
                        #      c0-live[768:896] dead[896:1024]
                        # spC: dead[0:128] c9-live[128:256] c8[256:512]
                        # -> each tile needs ONE contiguous exp op
                        spA = sp2p.tile([128, 1024], f32, tag="sp2", name="spA")
                        spB = sp2p.tile([128, 1024], f32, tag="sp2", name="spB")
                        spC = sp1p.tile([128, 512], f32, tag="spc", name="spC")
                        kcol = lambda jc: (2 * qb + jc) * 128
                        nc.tensor.matmul(
                            spA[:, 768:896],
                            R(kt[rb:rb + 64, kcol(0):kcol(0) + 128]),
                            R(qt[rb:rb + 64, q0:q0 + 128]),
                            start=True, stop=True)
                        for jc in range(1, 4):
                            nc.tensor.matmul(
                                spA[:, (jc - 1) * 256:jc * 256],
                                R(kt[rb:rb + 64, kcol(jc):kcol(jc) + 128]),
                                R(qt[rb:rb + 64, q0:q0 + 256]),
                                start=True, stop=True)
                        for jc in range(4, 8):
                            nc.tensor.matmul(
                                spB[:, (jc - 4) * 256:(jc - 3) * 256],
                                R(kt[rb:rb + 64, kcol(jc):kcol(jc) + 128]),
                                R(qt[rb:rb + 64, q0:q0 + 256]),
                                start=True, stop=True)
                        nc.tensor.matmul(
                            spC[:, 256:512],
                            R(kt[rb:rb + 64, kcol(8):kcol(8) + 128]),
                            R(qt[rb:rb + 64, q0:q0 + 256]),
                            start=True, stop=True)
                        nc.tensor.matmul(
                            spC[:, 128:256],
                            R(kt[rb:rb + 64, kcol(9):kcol(9) + 128]),
                            R(qt[rb:rb + 64, q0 + 128:q0 + 256]),
                            start=True, stop=True)
                        epA = epp.tile([128, 1024], f32r, tag="ep2", name="epA")
                        epB = epp.tile([128, 1024], f32r, tag="ep2", name="epB")
                        epC = epp.tile([128, 512], f32r, tag="epc", name="epC")
                        nc.scalar.activation(epA[:, 0:896], spA[:, 0:896],
                                             func=Act.Exp, scale=SCL)
                        nc.scalar.activation(epB, spB, func=Act.Exp, scale=SCL)
                        nc.scalar.activation(epC[:, 128:512], spC[:, 128:512],
                                             func=Act.Exp, scale=SCL)
                        nc.gpsimd.memset(epA[:, 896:1024].bitcast(f32), 0.0)
                        nc.gpsimd.memset(epC[:, 0:128].bitcast(f32), 0.0)
                        # masks: epA cols {128:256, 768:896} *= [T1|T1]
                        #        epC cols [128:384] *= [T2|T2]
                        eva = bass.AP(
                            tensor=epA.tensor, offset=epA.offset + 128,
                            ap=[epA.ap[0], [640, 2], [1, 128]])
                        tva = bass.AP(
                            tensor=t12_s.tensor, offset=t12_s.offset,
                            ap=[t12_s.ap[0], [128, 2], [1, 128]])
                        nc.vector.tensor_mul(eva, eva, tva)
                        nc.vector.tensor_mul(epC[:, 128:384], epC[:, 128:384],
                                             t12_s[:, 256:512])
                        # PV: plain N=256 accumulation chain over 10 chunks
                        vof = lambda jc: (2 * qb + jc) * VSTR + h * 66
                        for jc in range(10):
                            if jc == 0:
                                rhs = epA[:, 768:1024]
                            elif jc < 4:
                                rhs = epA[:, (jc - 1) * 256:jc * 256]
                            elif jc < 8:
                                rhs = epB[:, (jc - 4) * 256:(jc - 3) * 256]
                            elif jc == 8:
                                rhs = epC[:, 256:512]
                            else:
                                rhs = epC[:, 0:256]
                            nc.tensor.matmul(
                                yph[qb // 2][:, (qb % 2) * 256:(qb % 2) * 256 + 256],
                                R(v_sb[:, vof(jc):vof(jc) + 65]), R(rhs),
                                start=(jc == 0), stop=(jc == 9))
                        # per-block Z extraction (overlaps later blocks)
                        yp = yph[qb // 2]
                        p0 = (qb % 2) * 256
                        nc.vector.tensor_copy(zrow[:, q0:q0 + 256],
                                              yp[64:65, p0:p0 + 256])
                        zcq = sp1p.tile([128, 2], f32, tag="spc", name="zcq")
                        for sub in range(2):
                            cq = 2 * qb + sub
                            nc.tensor.matmul(
                                zcq[:, sub:sub + 1],
                                zrow[:, cq * 128:(cq + 1) * 128],
                                one11, start=True, stop=True)
                        nc.vector.tensor_copy(
                            zcolsb[:, 2 * qb:2 * qb + 2], zcq)
                        nc.vector.tensor_copy(ysb[:, q0:q0 + 256],
                                              yp[0:64, p0:p0 + 256])
                    # ---- head-tail normalization ----
                    zc = zp.tile([128, NQB], f32, tag="zc", name="zc")
                    nc.vector.scalar_tensor_tensor(
                        zc, zcolsb, 1.0, npad_s, op0=Alu.mult, op1=Alu.subtract)
                    zr = zp.tile([128, NQB], f32, tag="zr", name="zr")
                    nc.vector.reciprocal(zr, zc)
                    zrb = zp.tile([128, NQB * D], f32, tag="zrb", name="zrb")
                    for cq in range(NQB):
                        nc.vector.tensor_scalar_mul(
                            zrb[:, cq * D:(cq + 1) * D], ones64, zr[:, cq:cq + 1])
                    for half in range(2):
                        zbh = sp1p.tile([64, 512], f32, tag="spc", name="zbh")
                        for sub in range(4):
                            cq = half * 4 + sub
                            nc.tensor.matmul(
                                zbh[:, sub * 128:(sub + 1) * 128],
                                zrb[:, cq * D:(cq + 1) * D], idn,
                                start=True, stop=True)
                        nc.vector.scalar_tensor_tensor(
                            ya[ti][rb:rb + 64, half * 512:(half + 1) * 512],
                            ysb[:, half * 512:(half + 1) * 512], 1.0, zbh,
                            op0=Alu.mult, op1=Alu.mult)

            # ---- phase 4: output projection + bias + residual ----
            with tc.tile_pool(name="proj2", bufs=2, space="PSUM") as proj2:
                for m in range(2):
                    for n in range(2):
                        ps = proj2.tile([128, 512], f32, tag="proj2")
                        nc.tensor.matmul(ps, R(wp0[:, m * 128:(m + 1) * 128]),
                                         R(ya[0][:, n * 512:(n + 1) * 512]),
                                         start=True, stop=False)
                        nc.tensor.matmul(ps, R(wp1[:, m * 128:(m + 1) * 128]),
                                         R(ya[1][:, n * 512:(n + 1) * 512]),
                                         start=False, stop=True)
                        nc.vector.scalar_tensor_tensor(
                            ot[m][:, n * 512:(n + 1) * 512], ps,
                            beff_s[:, m:m + 1],
                            xe(m, TCH + n * 512, 512).bitcast(f32),
                            op0=Alu.add, op1=Alu.add)
                    nc.sync.dma_start(out=out[m * 128:(m + 1) * 128, :], in_=ot[m])
    nc.compile()
    return nc


_NC = None


def _get_nc():
    global _NC
    if _NC is None:
        _NC = _build_nc()
    return _NC


def _prepare_in_maps(x, W_qkv, b_qkv, W_proj, b_proj):
    x = np.asarray(x, dtype=np.float32)
    W_qkv = np.asarray(W_qkv, dtype=np.float32)
    b_qkv = np.asarray(b_qkv, dtype=np.float32)
    W_proj = np.asarray(W_proj, dtype=np.float32)
    b_proj = np.asarray(b_proj, dtype=np.float32)

    o1, o2, o3 = _B1OFF, _B2OFF, _B3OFF
    base1 = np.zeros((128, NB1), dtype=np.float32)
    base2 = np.zeros((128, NB2), dtype=np.float32)
    base3 = np.zeros((128, NB3), dtype=np.float32)

    wT = W_qkv[: 2 * C, :].T          # [256, 512]
    base1[:, o1["we0"]:o1["we0"] + 512] = wT[0:128]
    base1[:, o1["we1"]:o1["we1"] + 512] = wT[128:256]
    base1[0, o1["xvw"] + TLOC:o1["xvw"] + TLOC + 512] = b_qkv[: 2 * C]

    wv = np.zeros((C, VSTR), dtype=np.float32)
    for h in range(H):
        wv[:, h * 66:h * 66 + D] = W_qkv[2 * C + h * D: 2 * C + (h + 1) * D, :].T
    base2[:, o2["wv0"]:o2["wv0"] + VSTR] = wv[0:128]
    base2[:, o2["wv1"]:o2["wv1"] + VSTR] = wv[128:256]

    wpT = W_proj.T
    base2[:, o2["wp0"]:o2["wp0"] + C] = wpT[0:128]
    base2[:, o2["wp1"]:o2["wp1"] + C] = wpT[128:256]

    b_eff = b_proj + W_proj @ b_qkv[2 * C:]
    base3[:, o3["beff"]:o3["beff"] + 2] = b_eff.reshape(2, 128).T

    r = np.arange(128)[:, None]
    i = np.arange(128)[None, :]
    t1 = (r > i).astype(np.float32)
    t2 = (r <= i).astype(np.float32)
    base3[:, o3["t12"]:o3["t12"] + 128] = t1
    base3[:, o3["t12"] + 128:o3["t12"] + 256] = t1
    base3[:, o3["t12"] + 256:o3["t12"] + 384] = t2
    base3[:, o3["t12"] + 384:o3["t12"] + 512] = t2

    base3[:, o3["ident"]:o3["ident"] + 128] = np.eye(128, dtype=np.float32)

    npad0 = np.maximum(0.0, 1023.0 - np.arange(TCH, dtype=np.float32))
    npadT0 = npad0.reshape(NQB, 128).T

    in_maps = []
    for core in range(8):
        b = core // 4
        tci = core % 4
        b1 = base1.copy()
        b2 = base2.copy()
        b3 = base3.copy()
        xl = np.zeros((C, TLOC), dtype=np.float32)
        if tci == 0:
            xl[:, WINDOW:] = x[b, :, 0:TCH]
            b1[0, o1["xvw"] + WINDOW:o1["xvw"] + TLOC] = 1.0
            b3[:, o3["npadT"]:o3["npadT"] + NQB] = npadT0
        else:
            xl[:, :] = x[b, :, (tci - 1) * TCH:(tci + 1) * TCH]
            b1[0, o1["xvw"]:o1["xvw"] + TLOC] = 1.0
        b1[:, o1["xe0h"]:o1["xe0h"] + 1024] = xl[0:128, 1024:2048]
        b1[:, o1["xe1h"]:o1["xe1h"] + 1024] = xl[128:256, 1024:2048]
        b2[:, o2["xe0l"]:o2["xe0l"] + 1024] = xl[0:128, 0:1024]
        b2[:, o2["xe1l"]:o2["xe1l"] + 1024] = xl[128:256, 0:1024]
        in_maps.append({"blob1": b1, "blob2": b2, "blob3": b3})
    return in_maps


def _run(in_maps, trace=False):
    from concourse import bass_utils
    nc = _get_nc()
    return bass_utils.run_bass_kernel_spmd(
        nc, in_maps, core_ids=list(range(8)), trace=trace)


def kernel(x, W_qkv, b_qkv, W_proj, b_proj):
    in_maps = _prepare_in_maps(x, W_qkv, b_qkv, W_proj, b_proj)
    res = _run(in_maps, trace=False)
    out = np.zeros((B, C, T), dtype=np.float32)
    for core in range(8):
        b = core // 4
        tci = core % 4
        out[b, :, tci * TCH:(tci + 1) * TCH] = res.results[core]["out"]
    return out



# revision 6
# speedup vs baseline: 1.5210x; 1.5210x over previous
"""Trainium2 Bass kernel for causal sliding-window self-attention (v2).

Reference computation (per batch b):
    qkv = W_qkv @ x + b_qkv            # [3C, T], C=256, T=4096
    q,k,v -> [H=4, D=64, T]
    attn = softmax(mask(q^T k / sqrt(D)))   # causal, window 1024
    y = attn @ v                       # [H, D, T] -> [C, T]
    out = x + W_proj @ y + b_proj

Sharding: 8 cores = 2 batches x 4 time-chunks of 1024 queries; each core
gets its chunk plus a 1024-step key/value halo, computes all heads and the
full output projection locally; unshard is concatenation.

v2 design notes (cost-model-driven):
  - All activations bf16; psum accumulation f32. Max rel err ~3e-3.
  - Bias algebra: the K-bias term contributes a per-query constant to the
    scores, which cancels in softmax -> k projected with NO bias (2-chain
    matmuls). The Q-bias is applied during the psum->sbuf copy as a
    per-partition tensor_scalar_add. The V-bias passes through the
    softmax-average additively -> folded into the residual on the host
    (xres = x + b_proj + W_proj @ b_v).
  - Scores for a 128-query subblock sb live in ONE psum tile: 9 key chunks
    of 128 (window 1024 + current). The two triangular edge chunks get a
    -240 additive mask via chained bf16 matmuls against constant
    triangle/identity operands (exp -> 0 exactly).
  - One exp per subblock: activation [128,1152] psum->sbuf bf16 (the Act
    engine is the roofline: 32 * 1152 cols).
  - PV runs TRANSPOSED: yT[t,65] = sum_s ep[s,t] * v[s, d|ones] with ep
    chunks as the stationary operand -> out free size 65 instead of 256.
    Column 64 accumulates Z[t] via a host-supplied valid-column in v
    (zero for halo padding, so no pad-count corrections anywhere).
  - Normalization: reciprocal of Z is a per-partition scalar in the
    transposed layout; tensor_scalar_mul writes normalized bf16. Pairs of
    subblocks are transposed back to [d, t] by PE transposes against a
    constant identity, one DVE copy per pair into quarter-sized ya tiles.
  - Output projection: 2-chain bf16 matmuls; residual+bias via one DVE
    tensor_add against the host-prepared xres; DMA out.
"""

import math
import sys

import numpy as np

if "/opt/trn_rl_repo" not in sys.path:
    sys.path.insert(0, "/opt/trn_rl_repo")

import ml_dtypes

B = 2
C = 256
T = 4096
H = 4
D = 64
WINDOW = 1024
TCH = 1024          # queries per core
TLOC = 2048         # halo + chunk
NSB = TCH // 128    # 8 query subblocks
NVCH = TLOC // 128  # 16 v chunks
VSTR = 4 * 66       # v chunk stride: 4 heads x (64 d + 1 valid + 1 pad)
NEG = -240.0        # triangle mask add (exp -> 0)

SCL = 1.0 / math.sqrt(D)

BF16 = ml_dtypes.bfloat16
F8 = ml_dtypes.float8_e4m3

# bf16 blob layout (columns)
XB0 = 0                    # [128, 2, 2048] x local, c-chunk-major
XR0 = XB0 + 2 * TLOC       # [128, 2, 1024] x own + beff
VV0 = XR0 + 2 * TCH        # [128, 64] v valid cols (chunk-major x 4 heads)
WQK0 = VV0 + 64            # [128, 2, 512]: i*512 + (q 0:256 | k 256:512)
WV0 = WQK0 + 1024          # [128, 2, 264]
WP0 = WV0 + 2 * VSTR       # [128, 2, 256]
ID0 = WP0 + 512            # [128, 128] bf16 identity (PE transpose ifmap)
TB0 = ID0 + 128            # [128, 2*128] bf16 triangle adds (Mw^T, Mc^T)
NBB = TB0 + 256

SCW = 1152                 # score tile cols (3 psum banks)


def _build_nc():
    import concourse.bass as bass
    import concourse.bacc as bacc
    import concourse.tile as tile
    from concourse import mybir

    f32 = mybir.dt.float32
    bf16 = mybir.dt.bfloat16
    f8 = mybir.dt.float8e4
    Act = mybir.ActivationFunctionType
    DR = mybir.MatmulPerfMode.DoubleRow

    nc = bacc.Bacc("TRN2", debug=False, target_bir_lowering=False, num_devices=8)

    bb_d = nc.dram_tensor("bb", [128, NBB], bf16, kind="ExternalInput").ap()
    bf_d = nc.dram_tensor("bf", [128, 2], f32, kind="ExternalInput").ap()
    out_d = nc.dram_tensor("out", [C, TCH], f32, kind="ExternalOutput").ap()

    with tile.TileContext(nc) as tc:
        with tc.tile_pool(name="sing", bufs=1) as sing:
            bb = sing.tile([128, NBB], bf16)
            bqc = sing.tile([128, 2], f32)
            qs = [sing.tile([128, TCH], bf16, name=f"qs{i}") for i in range(2)]
            ks = [sing.tile([128, TLOC], bf16, name=f"ks{i}") for i in range(2)]
            vsb = sing.tile([128, NVCH * VSTR], bf16)
            # ya split into column-quarter tiles: decouples transpose writes
            # from outproj reads (dep tracking is per-tile)
            yaq = [[sing.tile([128, 256], bf16, name=f"ya{i}_{q}")
                    for q in range(4)] for i in range(2)]
            ot = [sing.tile([128, TCH], f32, name=f"ot{i}") for i in range(2)]
            wrm = sing.tile([64, 512], bf16)

            def bbv(c0, dims):
                # view into bb at column c0 with free dims `dims`
                return bass.AP(tensor=bb.tensor, offset=bb.offset + c0,
                               ap=[bb.ap[0]] + dims)

            # ---- warmup: PE p-state ramp + Act table load ----
            nc.gpsimd.memset(wrm, 0.0)
            wex = sing.tile([1, 4], bf16)
            nc.scalar.activation(wex, wrm[0:1, 0:4], func=Act.Exp, scale=1.0)

            # ---- DMA loads (ordered by first use; xres loaded late) ----
            nc.sync.dma_start(out=bb[:, VV0:WQK0 + 1024],
                              in_=bb_d[:, VV0:WQK0 + 1024])
            xb_dmas = []
            for tb in range(4):
                dst = bbv(XB0 + tb * 512, [[TLOC, 2], [1, 512]])
                src = bass.AP(tensor=bb_d.tensor, offset=bb_d.offset + XB0 + tb * 512,
                              ap=[bb_d.ap[0], [TLOC, 2], [1, 512]])
                xb_dmas.append((dst, src))
            nc.sync.dma_start(out=xb_dmas[0][0], in_=xb_dmas[0][1])
            nc.sync.dma_start(out=xb_dmas[1][0], in_=xb_dmas[1][1])
            nc.sync.dma_start(out=xb_dmas[2][0], in_=xb_dmas[2][1])
            nc.sync.dma_start(out=bb[:, WV0:NBB], in_=bb_d[:, WV0:NBB])
            nc.sync.dma_start(out=bqc, in_=bf_d)
            nc.sync.dma_start(out=xb_dmas[3][0], in_=xb_dmas[3][1])
            # v valid columns: bb[VV0 + jc*4 + h] -> vsb[jc*264 + h*66 + 64]
            vv_src = bbv(VV0, [[4, NVCH], [1, 4]])
            vv_dst = bass.AP(tensor=vsb.tensor, offset=vsb.offset + 64,
                             ap=[vsb.ap[0], [VSTR, NVCH], [66, 4]])
            nc.vector.tensor_copy(vv_dst, vv_src)

            with tc.tile_pool(name="wp0", bufs=1, space="PSUM") as wp0:
                jp = wp0.tile([128, 512], f32)
                for _ in range(3):
                    nc.tensor.matmul(jp, wrm[:, 0:128], wrm, start=True, stop=True)

            # ---- projection helpers ----
            def xbv(i, c0, w):
                return bbv(XB0 + i * TLOC + c0, [[1, w]])

            def kproj(pool, m, tb):
                ps = pool.tile([128, 512], f32, tag="u", bufs=2, name="kps")
                for i in range(2):
                    st = bbv(WQK0 + i * 512 + 256 + m * 128, [[1, 128]])
                    nc.tensor.matmul(ps, st, xbv(i, tb * 512, 512),
                                     start=(i == 0), stop=(i == 1))
                nc.vector.tensor_copy(ks[m][:, tb * 512:(tb + 1) * 512], ps)

            def qproj(pool, m, tb):
                ps = pool.tile([128, 512], f32, tag="u", bufs=2, name="qps")
                for i in range(2):
                    st = bbv(WQK0 + i * 512 + m * 128, [[1, 128]])
                    nc.tensor.matmul(ps, st, xbv(i, TCH + tb * 512, 512),
                                     start=(i == 0), stop=(i == 1))
                nc.vector.tensor_scalar_add(qs[m][:, tb * 512:(tb + 1) * 512],
                                            ps, bqc[:, m:m + 1])

            vcnt = [0]

            def vproj2(pool, jp):
                # two v chunks (2*jp, 2*jp+1), copies alternate DVE/Pool
                for half in range(2):
                    jc = 2 * jp + half
                    ps = pool.tile([128, 512], f32, tag="u", bufs=2, name="vps")
                    for i in range(2):
                        st = bbv(XB0 + i * TLOC + jc * 128, [[1, 128]])
                        nc.tensor.matmul(ps[:, 0:VSTR], st,
                                         bbv(WV0 + i * VSTR, [[1, VSTR]]),
                                         start=(i == 0), stop=(i == 1))
                    dst = bass.AP(tensor=vsb.tensor,
                                  offset=vsb.offset + jc * VSTR,
                                  ap=[vsb.ap[0], [66, 4], [1, 64]])
                    src = bass.AP(tensor=ps.tensor, offset=ps.offset,
                                  ap=[ps.ap[0], [66, 4], [1, 64]])
                    vcnt[0] += 1
                    nc.vector.tensor_copy(dst, src)


            # ---- phase 2: attention (PV tail pipelined one subblock back) ----
            gaps = {
                (0, 2): [("k", 0, 3), ("v", 5, 0)],
                (0, 3): [("q", 0, 1)],
                (0, 4): [("v", 6, 0)],
                (0, 6): [("v", 7, 0)],
                (1, 0): [("k", 1, 0)],
                (1, 1): [("k", 1, 1)],
                (1, 2): [("k", 1, 2)],
                (1, 3): [("k", 1, 3)],
                (1, 4): [("q", 1, 0)],
                (1, 5): [("q", 1, 1)],
            }
            # h0, h1 in order (late projections in the gaps); then h2/h3
            # interleaved by subblock pair so ya fills column-major and the
            # first output-projection half can overlap the attention tail.
            seq = [(0, sb) for sb in range(NSB)] + [(1, sb) for sb in range(NSB)]
            for pp2 in range(4):
                seq += [(2, 2 * pp2), (2, 2 * pp2 + 1),
                        (3, 2 * pp2), (3, 2 * pp2 + 1)]

            def scores(scp, sbp, h, sb):
                ti, rb = h // 2, (h % 2) * 64
                sc = scp.tile([128, SCW], f32, tag="sc", bufs=2, name="sc")
                assert sc is not None
                for c in range(9):
                    ch = sb + c
                    tri = c == 0 or c == 8
                    nc.tensor.matmul(
                        sc[:, c * 128:(c + 1) * 128],
                        ks[ti][rb:rb + 64, ch * 128:(ch + 1) * 128],
                        qs[ti][rb:rb + 64, sb * 128:(sb + 1) * 128],
                        start=True, stop=not tri)
                    if tri:
                        tb_ = bbv(TB0 + (0 if c == 0 else 128), [[1, 128]])
                        nc.tensor.matmul(
                            sc[:, c * 128:(c + 1) * 128],
                            tb_, bbv(ID0, [[1, 128]]),
                            start=False, stop=True)
                ep = sbp.tile([128, 1152], bf16, tag="ep", bufs=3, name="ep")
                nc.scalar.activation(ep, sc[:, 0:1152], func=Act.Exp, scale=SCL)
                return sc, ep

            def tail(sbp, misc, h, sb, ep, ysb):
                # yt has its own psum ring (shared with the late-projection
                # psum slots) so score buffers recycle on the exp read alone.
                ti, rb = h // 2, (h % 2) * 64
                ytt = misc.tile([128, 512], f32, tag="u", bufs=2, name="ytt")
                yt = ytt[:, 0:65]
                for c in range(9):
                    ch = sb + c
                    nc.tensor.matmul(
                        yt, ep[:, c * 128:(c + 1) * 128],
                        vsb[:, ch * VSTR + h * 66:ch * VSTR + h * 66 + 65],
                        start=(c == 0), stop=(c == 8))
                zr = sbp.tile([128, 1], f32, tag="zr", bufs=2, name="zr")
                nc.vector.reciprocal(zr, yt[:, 64:65])
                half = sb % 2
                nc.vector.tensor_scalar_mul(
                    ysb[:, half * 64:half * 64 + 64], yt[:, 0:64], zr)
                if half == 1:
                    # PE transpose back to [d, t] + one DVE copy per pair
                    tp = misc.tile([128, 256], bf16, tag="u", bufs=2, name="tp")
                    ident = bbv(ID0, [[1, 128]])
                    for i2 in range(2):
                        nc.tensor.transpose(
                            tp[rb:rb + 64, i2 * 128:(i2 + 1) * 128],
                            ysb[:, i2 * 64:(i2 + 1) * 64], ident)
                    nc.vector.tensor_copy(
                        yaq[ti][sb // 2][rb:rb + 64, 0:256],
                        tp[rb:rb + 64, 0:256])

            def outproj(outp, m, q):
                # quarter granularity (256 cols) so all but the last quarter
                # overlap the attention tail
                c0 = q * 256
                ps = outp.tile([128, 512], f32, tag="u", bufs=2, name="ops")
                for i in range(2):
                    st = bbv(WP0 + i * 256 + m * 128, [[1, 128]])
                    nc.tensor.matmul(ps[:, 0:256], st, yaq[i][q],
                                     start=(i == 0), stop=(i == 1))
                xr = bbv(XR0 + m * TCH + c0, [[1, 256]])
                nc.vector.tensor_add(ot[m][:, c0:c0 + 256], ps[:, 0:256], xr)
                nc.sync.dma_start(
                    out=out_d[m * 128:(m + 1) * 128, c0:c0 + 256],
                    in_=ot[m][:, c0:c0 + 256])

            with tc.tile_pool(name="scp", bufs=1, space="PSUM") as scp, \
                 tc.tile_pool(name="misc", bufs=1, space="PSUM") as misc, \
                 tc.tile_pool(name="sbp", bufs=1) as sbp:
                for tb in range(3):
                    kproj(misc, 0, tb)
                qproj(misc, 0, 0)
                ysbs = {}
                # head: two subblocks of scores/exp before the v chain so the
                # Act pipeline starts as early as the q/k projections allow
                ysbs[0] = sbp.tile([128, 128], bf16, tag="ysb", bufs=3,
                                   name="ysb")
                pending_q = None
                for jp in range(2):
                    vproj2(misc, jp)
                sc0, ep0 = scores(scp, sbp, 0, 0)
                for jp in range(2, 4):
                    vproj2(misc, jp)
                sc1, ep1 = scores(scp, sbp, 0, 1)
                vproj2(misc, 4)
                tail(sbp, misc, 0, 0, ep0, ysbs[0])
                prev = (0, 1, ep1, ysbs[0])
                for h, sb in seq[2:]:
                    if sb % 2 == 0:
                        ysbs[h] = sbp.tile([128, 128], bf16, tag="ysb", bufs=3,
                                           name="ysb")
                    if pending_q is not None:
                        for m in range(2):
                            outproj(misc, m, pending_q)
                        pending_q = None
                    sc, ep = scores(scp, sbp, h, sb)
                    done = None
                    if prev is not None:
                        tail(sbp, misc, *prev)
                        done = (prev[0], prev[1])
                    prev = (h, sb, ep, ysbs[h])
                    for kind, a, b2 in gaps.get((h, sb), []):
                        if kind == "k":
                            kproj(misc, a, b2)
                        elif kind == "q":
                            qproj(misc, a, b2)
                        else:
                            vproj2(misc, a)
                    if (h, sb) == (1, NSB - 1):
                        nc.sync.dma_start(out=bb[:, XR0:XR0 + 2 * TCH],
                                          in_=bb_d[:, XR0:XR0 + 2 * TCH])
                    if done is not None and done[0] == 3 and done[1] % 2 == 1 \
                            and done[1] < NSB - 1:
                        pending_q = done[1] // 2
                tail(sbp, misc, *prev)
                for m in range(2):
                    outproj(misc, m, 3)
    nc.compile()
    return nc


_NC = None


def _get_nc():
    global _NC
    if _NC is None:
        _NC = _build_nc()
    return _NC


def _prepare_in_maps(x, W_qkv, b_qkv, W_proj, b_proj):
    x = np.asarray(x, dtype=np.float32)
    W_qkv = np.asarray(W_qkv, dtype=np.float32)
    b_qkv = np.asarray(b_qkv, dtype=np.float32)
    W_proj = np.asarray(W_proj, dtype=np.float32)
    b_proj = np.asarray(b_proj, dtype=np.float32)

    base = np.zeros((128, NBB), dtype=np.float32)
    # wqk: [p, i*512 + o]: o<256 -> Wq^T, o>=256 -> Wk^T (channel i*128+p)
    for i in range(2):
        base[:, WQK0 + i * 512:WQK0 + i * 512 + 256] = \
            W_qkv[:C, i * 128:(i + 1) * 128].T
        base[:, WQK0 + i * 512 + 256:WQK0 + (i + 1) * 512] = \
            W_qkv[C:2 * C, i * 128:(i + 1) * 128].T
    # wv: [p, i*264 + h*66 + d]
    for i in range(2):
        for h in range(H):
            base[:, WV0 + i * VSTR + h * 66:WV0 + i * VSTR + h * 66 + D] = \
                W_qkv[2 * C + h * D:2 * C + (h + 1) * D, i * 128:(i + 1) * 128].T
    # wp: [p, i*256 + o]
    for i in range(2):
        base[:, WP0 + i * 256:WP0 + (i + 1) * 256] = \
            W_proj[:, i * 128:(i + 1) * 128].T
    base[:, ID0:ID0 + 128] = np.eye(128, dtype=np.float32)
    s_i = np.arange(128)[:, None]
    t_i = np.arange(128)[None, :]
    base[:, TB0:TB0 + 128] = np.where(s_i <= t_i, NEG, 0.0).T
    base[:, TB0 + 128:TB0 + 256] = np.where(s_i > t_i, NEG, 0.0).T

    beff = b_proj + W_proj @ b_qkv[2 * C:]

    bf = np.zeros((128, 2), dtype=np.float32)
    bf[:, 0] = b_qkv[0:128]
    bf[:, 1] = b_qkv[128:256]

    in_maps = []
    for core in range(8):
        b = core // 4
        tci = core % 4
        bb = base.copy()
        xl = np.zeros((C, TLOC), dtype=np.float32)
        if tci == 0:
            xl[:, WINDOW:] = x[b, :, 0:TCH]
            valid0 = 0.0
        else:
            xl[:, :] = x[b, :, (tci - 1) * TCH:(tci + 1) * TCH]
            valid0 = 1.0
        for i in range(2):
            bb[:, XB0 + i * TLOC:XB0 + (i + 1) * TLOC] = xl[i * 128:(i + 1) * 128]
            bb[:, XR0 + i * TCH:XR0 + (i + 1) * TCH] = \
                xl[i * 128:(i + 1) * 128, TCH:] + beff[i * 128:(i + 1) * 128, None]
        vv = np.ones((NVCH, 4), dtype=np.float32)
        vv[0:8] = valid0
        bb[:, VV0:VV0 + 64] = vv.reshape(-1)[None, :]
        in_maps.append({"bb": bb.astype(BF16), "bf": bf})
    return in_maps


def _run(in_maps, trace=False):
    from concourse import bass_utils
    nc = _get_nc()
    return bass_utils.run_bass_kernel_spmd(
        nc, in_maps, core_ids=list(range(8)), trace=trace)


def kernel(x, W_qkv, b_qkv, W_proj, b_proj):
    in_maps = _prepare_in_maps(x, W_qkv, b_qkv, W_proj, b_proj)
    res = _run(in_maps, trace=False)
    out = np.zeros((B, C, T), dtype=np.float32)
    for core in range(8):
        b = core // 4
        tci = core % 4
        out[b, :, tci * TCH:(tci + 1) * TCH] = res.results[core]["out"]
    return out
